# revision 3
# baseline (speedup 1.0000x reference)
"""Trainium2 Bass kernel for nn_DiagnosticNet (topk_masking).

Math (per row b of x [B, D]):
    s[b]     = x[b] @ enc_W.T + enc_b                  # [K]
    idx[b]   = argmax(s[b])
    y_hat[b] = dot(dec_W[0][idx[b]], x[b]) + dec_bias[idx[b]]
    loss     = mean((y_hat - y)^2); accuracy = mean(sign(y_hat) == y)

Strategy: data-parallel over B across 8 cores. On each core, one fused
matmul chain per 128-row tile computes [s | t] where t = x @ dec_W[0].T,
then a row-wise argmax + one-hot select picks y_hat = t[idx] on-chip.
Precision: x is shipped as a bf16 hi/lo pair (same bytes as f32); s uses
the 3-term product (hi*Wh + lo*Wh + hi*Wl) which matches fp32 matmul
error to ~8e-6 at 2x the speed of native fp32 matmuls; t uses hi*Wh
(bf16-level error, fine for y_hat magnitudes). Weights stay replicated.
"""

import sys

import numpy as np

sys.path.insert(0, "/opt/trn_rl_repo")

import ml_dtypes  # noqa: E402
import concourse.bass as bass  # noqa: E402
import concourse.mybir as mybir  # noqa: E402
import concourse.tile as tile  # noqa: E402
from concourse.bass_utils import run_bass_kernel_spmd  # noqa: E402
from concourse.vector_clock import ScopedClock  # noqa: E402

B, D, K = 16384, 2048, 128
NCORES = 8
BS = B // NCORES  # rows per core
NBT = BS // 128  # 128-row tiles per core
ND = D // 128  # 128-deep contraction chunks
BW = 512  # staged batch width (4 tiles per stage)
NST = BS // BW  # stages per core

dt = mybir.dt


def _split_waits(inst):
    """This toolchain's walrus accepts at most one sync-wait command per
    instruction; return carrier NOPs for the excess waits."""
    si = inst.sync_info
    if si is None:
        return []
    waits = si.on_wait
    if not waits or len(waits) <= 1:
        return []
    extras = list(waits[:-1])
    si.on_wait = [waits[-1]]
    nops = []
    for k, w in enumerate(extras):
        nop = mybir.InstNoOp(name=f"{inst.name}-swait{k}", ins=[], outs=[])
        nop.engine = inst.engine
        nop.sync_info = mybir.SyncInfo(on_wait=[w], on_update=[])
        nops.append(nop)
    return nops


class TileContextFixed(tile.TileContext):
    def _add_instruction(self, inst):
        for nop in _split_waits(inst):
            super()._add_instruction(nop)
        super()._add_instruction(inst)

    def _drain_and_barrier(self, tick_clock, wait_clock):
        nc = self.nc
        drain_inst = nc.sync.drain()
        wait_clock.add_sem_waits(
            drain_inst.ins, ScopedClock({None: tick_clock.global_clock})
        )
        si = drain_inst.ins.sync_info
        waits = list(si.on_wait) if si and si.on_wait else []
        if len(waits) > 1:
            si.on_wait = [waits[0]]
            for k, w in enumerate(waits[1:]):
                nop = mybir.InstNoOp(
                    name=f"{drain_inst.ins.name}-dwait{k}", ins=[], outs=[]
                )
                nop.engine = drain_inst.ins.engine
                nop.sync_info = mybir.SyncInfo(on_wait=[w], on_update=[])
                self._add_instruction(nop)
        nc.all_engine_barrier()
        assert self.sems is not None
        popped = nc._tile_sem_poison_stack.pop()
        assert popped is self._sem_poison
        nc.clear_and_free_semaphores(list(self.sems.allocated().values()))
        nc.all_engine_barrier()


def _build_nc():
    nc = bass.Bass("TRN2", target_bir_lowering=False, debug=False)

    xh_d = nc.dram_tensor("xh", [D, BS], dt.bfloat16, kind="ExternalInput")
    xl_d = nc.dram_tensor("xl", [D, BS], dt.bfloat16, kind="ExternalInput")
    # W columns: 0:128 enc_W hi | 128:256 dec_W hi | 256:384 enc_W lo
    w_d = nc.dram_tensor("w", [D, 384], dt.bfloat16, kind="ExternalInput")
    bias_d = nc.dram_tensor("bias", [128, 256], dt.float32, kind="ExternalInput")
    iota_d = nc.dram_tensor("iota", [128, 128], dt.float32, kind="ExternalInput")
    y_d = nc.dram_tensor("yin", [128, NBT], dt.float32, kind="ExternalInput")

    yhat_d = nc.dram_tensor("yhat", [128, NBT], dt.float32, kind="ExternalOutput")
    idx_d = nc.dram_tensor("idx", [128, NBT], dt.int32, kind="ExternalOutput")
    la_d = nc.dram_tensor("lossacc", [128, 2], dt.float32, kind="ExternalOutput")

    with TileContextFixed(nc) as tc:
        with (
            tc.tile_pool(name="const", bufs=1) as cpool,
            tc.tile_pool(name="xstage", bufs=2) as xpool,
            tc.tile_pool(name="ps", bufs=4, space="PSUM") as pspool,
            tc.tile_pool(name="work", bufs=4) as wpool,
            tc.tile_pool(name="outs", bufs=1) as opool,
        ):
            # ---- constants / weights ----
            w_t = cpool.tile([128, ND * 384], dt.bfloat16, tag="w")
            nc.sync.dma_start(
                w_t[:].rearrange("p (n k) -> p n k", n=ND),
                w_d.ap().rearrange("(n p) k -> p n k", p=128),
            )
            bias_t = cpool.tile([128, 256], dt.float32, tag="bias")
            nc.sync.dma_start(bias_t[:], bias_d.ap()[:])
            iota_t = cpool.tile([128, 128], dt.float32, tag="iota")
            nc.sync.dma_start(iota_t[:], iota_d.ap()[:])
            y_t = cpool.tile([128, NBT], dt.float32, tag="y")
            nc.sync.dma_start(y_t[:], y_d.ap()[:])

            yhat_t = opool.tile([128, NBT], dt.float32, tag="yhat")
            idx_t = opool.tile([128, NBT], dt.int32, tag="idx")
            la_t = opool.tile([128, 2], dt.float32, tag="la")

            def wsl(di, c0, c1):
                return w_t[:, di * 384 + c0 : di * 384 + c1]

            for st in range(NST):
                # ---- stage in BW batch columns of x (hi and lo) ----
                xh_t = xpool.tile([128, ND * BW], dt.bfloat16, tag="xh")
                nc.sync.dma_start(
                    xh_t[:].rearrange("p (n b) -> p n b", n=ND),
                    xh_d.ap().rearrange("(n p) b -> p n b", p=128)[
                        :, :, st * BW : (st + 1) * BW
                    ],
                )
                xl_t = xpool.tile([128, ND * BW], dt.bfloat16, tag="xl")
                nc.sync.dma_start(
                    xl_t[:].rearrange("p (n b) -> p n b", n=ND),
                    xl_d.ap().rearrange("(n p) b -> p n b", p=128)[
                        :, :, st * BW : (st + 1) * BW
                    ],
                )
                for bj in range(BW // 128):
                    bt = st * (BW // 128) + bj

                    def xsl(xt, di):
                        return xt[:, di * BW + bj * 128 : di * BW + (bj + 1) * 128]

                    ps = pspool.tile([128, 256], dt.float32, tag="ps")
                    n_mm = 3 * ND
                    i = 0
                    for di in range(ND):
                        # hi @ [Wh_enc | Wh_dec]  (N=256), then hi @ Wl_enc
                        # reusing the same stationary tile (N=128)
                        nc.tensor.matmul(
                            ps[:, 0:256], xsl(xh_t, di), wsl(di, 0, 256),
                            start=(i == 0), stop=False,
                        )
                        i += 1
                        nc.tensor.matmul(
                            ps[:, 0:128], xsl(xh_t, di), wsl(di, 256, 384),
                            start=False, stop=False,
                        )
                        i += 1
                        nc.tensor.matmul(
                            ps[:, 0:128], xsl(xl_t, di), wsl(di, 0, 128),
                            start=False, stop=(i == n_mm - 1),
                        )
                        i += 1

                    # ---- row-wise argmax + select ----
                    stt = wpool.tile([128, 256], dt.float32, tag="stt")
                    nc.vector.tensor_add(stt[:], ps[:], bias_t[:])
                    m8 = wpool.tile([128, 8], dt.float32, tag="m8")
                    nc.vector.max(out=m8[:], in_=stt[:, 0:128])
                    idx8 = wpool.tile([128, 8], dt.uint32, tag="idx8")
                    nc.vector.max_index(out=idx8[:], in_max=m8[:], in_values=stt[:, 0:128])
                    idxf = wpool.tile([128, 1], dt.float32, tag="idxf")
                    nc.vector.tensor_copy(idxf[:], idx8[:, 0:1])
                    onehot = wpool.tile([128, 128], dt.float32, tag="onehot")
                    nc.vector.tensor_scalar(
                        onehot[:], iota_t[:], idxf[:], None,
                        op0=mybir.AluOpType.is_equal,
                    )
                    prod = wpool.tile([128, 128], dt.float32, tag="prod")
                    nc.vector.tensor_mul(prod[:], onehot[:], stt[:, 128:256])
                    nc.vector.reduce_sum(
                        yhat_t[:, bt : bt + 1], prod[:], axis=mybir.AxisListType.X
                    )
                    nc.vector.tensor_copy(idx_t[:, bt : bt + 1], idx8[:, 0:1])

            # ---- loss / accuracy partials ----
            dd = wpool.tile([128, NBT], dt.float32, tag="dd")
            nc.vector.tensor_sub(dd[:], yhat_t[:], y_t[:])
            nc.vector.tensor_mul(dd[:], dd[:], dd[:])
            nc.vector.reduce_sum(la_t[:, 0:1], dd[:], axis=mybir.AxisListType.X)
            sg = wpool.tile([128, NBT], dt.float32, tag="sg")
            nc.vector.tensor_mul(sg[:], yhat_t[:], y_t[:])
            nc.vector.tensor_scalar(
                sg[:], sg[:], 0.0, None, op0=mybir.AluOpType.is_gt
            )
            nc.vector.reduce_sum(la_t[:, 1:2], sg[:], axis=mybir.AxisListType.X)

            nc.sync.dma_start(yhat_d.ap()[:], yhat_t[:])
            nc.sync.dma_start(idx_d.ap()[:], idx_t[:])
            nc.sync.dma_start(la_d.ap()[:], la_t[:])
    return nc


_NC_CACHE = []


def _get_nc():
    if not _NC_CACHE:
        _NC_CACHE.append(_build_nc())
    return _NC_CACHE[0]


def kernel(x, y, z, enc_W, enc_b, dec_W, dec_bias, _run_kwargs=None):
    x = np.ascontiguousarray(np.asarray(x, dtype=np.float32))
    y = np.asarray(y, dtype=np.float32)
    enc_W = np.asarray(enc_W, dtype=np.float32)
    enc_b = np.asarray(enc_b, dtype=np.float32)
    dec_W0 = np.asarray(dec_W, dtype=np.float32)[0]
    dec_bias = np.asarray(dec_bias, dtype=np.float32)

    # ---- host-side layout prep ----
    WT = np.concatenate([enc_W, dec_W0], axis=0).T  # [D, 256] f32
    WTh = WT.astype(ml_dtypes.bfloat16)
    WTl = (WT - WTh.astype(np.float32)).astype(ml_dtypes.bfloat16)
    wfull = np.concatenate([WTh, WTl[:, 0:128]], axis=1)  # [D, 384]
    wfull = np.ascontiguousarray(wfull)

    bias_rep = np.tile(
        np.concatenate([enc_b, dec_bias])[None, :], (128, 1)
    ).astype(np.float32)
    iota_np = np.tile(np.arange(128, dtype=np.float32)[None, :], (128, 1))

    in_maps = []
    for c in range(NCORES):
        xT = x[c * BS : (c + 1) * BS].T  # [D, BS] view
        xh = np.ascontiguousarray(xT.astype(ml_dtypes.bfloat16))
        xl = np.ascontiguousarray(
            (xT - xh.astype(np.float32)).astype(ml_dtypes.bfloat16)
        )
        ysh = np.ascontiguousarray(
            y[c * BS : (c + 1) * BS].reshape(NBT, 128).T
        )
        in_maps.append(
            {
                "xh": xh,
                "xl": xl,
                "w": wfull,
                "bias": bias_rep,
                "iota": iota_np,
                "yin": ysh,
            }
        )

    nc = _get_nc()
    res = run_bass_kernel_spmd(
        nc, in_maps, core_ids=list(range(NCORES)), **(_run_kwargs or {})
    )

    y_hat = np.empty(B, dtype=np.float32)
    z_hat = np.empty(B, dtype=np.int32)
    loss_sum = 0.0
    acc_cnt = 0.0
    for c in range(NCORES):
        out = res.results[c]
        y_hat[c * BS : (c + 1) * BS] = out["yhat"].T.reshape(BS)
        z_hat[c * BS : (c + 1) * BS] = out["idx"].T.reshape(BS)
        loss_sum += float(out["lossacc"][:, 0].sum())
        acc_cnt += float(out["lossacc"][:, 1].sum())
    loss = np.float32(loss_sum / B)
    accuracy = np.float32(acc_cnt / B)
    if _run_kwargs is not None:
        kernel.last_result = res
    return (y_hat, z_hat, loss, accuracy)


# revision 7
# speedup vs baseline: 19047.8714x; 19047.8714x over previous
"""Trainium2 Bass kernel for nn_DiagnosticNet (topk_masking).

Math (per row b of x [B, D]):
    s[b]     = x[b] @ enc_W.T + enc_b                  # [K]
    idx[b]   = argmax(s[b])
    y_hat[b] = dot(dec_W[0][idx[b]], x[b]) + dec_bias[idx[b]]
    loss     = mean((y_hat - y)^2); accuracy = mean(sign(y_hat) == y)

Strategy: data-parallel over B across 8 cores. On each core, one fused
matmul chain per 128-row tile computes [s | t] where t = x @ dec_W[0].T,
then a row-wise argmax + one-hot select picks y_hat = t[idx] on-chip.
Precision: x is shipped as a bf16 hi/lo pair (same bytes as f32); s uses
the 3-term product (hi*Wh + lo*Wh + hi*Wl) which matches fp32 matmul
error to ~8e-6 at 2x the speed of native fp32 matmuls; t uses hi*Wh
(bf16-level error, fine for y_hat magnitudes). Weights stay replicated.
"""

import sys

import numpy as np

sys.path.insert(0, "/opt/trn_rl_repo")

import ml_dtypes  # noqa: E402
import concourse.bass as bass  # noqa: E402
import concourse.mybir as mybir  # noqa: E402
import concourse.tile as tile  # noqa: E402
from concourse.bass_utils import run_bass_kernel_spmd  # noqa: E402
from concourse.vector_clock import ScopedClock  # noqa: E402

B, D, K = 16384, 2048, 128
NCORES = 8
BS = B // NCORES  # rows per core
NBT = BS // 128  # 128-row tiles per core
ND = D // 128  # 128-deep contraction chunks
BW = 512  # staged batch width (4 tiles per stage)
NST = BS // BW  # stages per core

dt = mybir.dt


def _split_waits(inst):
    """This toolchain's walrus accepts at most one sync-wait command per
    instruction; return carrier NOPs for the excess waits."""
    si = inst.sync_info
    if si is None:
        return []
    waits = si.on_wait
    if not waits or len(waits) <= 1:
        return []
    extras = list(waits[:-1])
    si.on_wait = [waits[-1]]
    nops = []
    for k, w in enumerate(extras):
        nop = mybir.InstNoOp(name=f"{inst.name}-swait{k}", ins=[], outs=[])
        nop.engine = inst.engine
        nop.sync_info = mybir.SyncInfo(on_wait=[w], on_update=[])
        nops.append(nop)
    return nops


class TileContextFixed(tile.TileContext):
    def _add_instruction(self, inst):
        for nop in _split_waits(inst):
            super()._add_instruction(nop)
        super()._add_instruction(inst)

    def _drain_and_barrier(self, tick_clock, wait_clock):
        nc = self.nc
        drain_inst = nc.sync.drain()
        wait_clock.add_sem_waits(
            drain_inst.ins, ScopedClock({None: tick_clock.global_clock})
        )
        si = drain_inst.ins.sync_info
        waits = list(si.on_wait) if si and si.on_wait else []
        if len(waits) > 1:
            si.on_wait = [waits[0]]
            for k, w in enumerate(waits[1:]):
                nop = mybir.InstNoOp(
                    name=f"{drain_inst.ins.name}-dwait{k}", ins=[], outs=[]
                )
                nop.engine = drain_inst.ins.engine
                nop.sync_info = mybir.SyncInfo(on_wait=[w], on_update=[])
                self._add_instruction(nop)
        nc.all_engine_barrier()
        assert self.sems is not None
        popped = nc._tile_sem_poison_stack.pop()
        assert popped is self._sem_poison
        nc.clear_and_free_semaphores(list(self.sems.allocated().values()))
        nc.all_engine_barrier()


def _build_nc(reps=1):
    """reps>1 repeats the whole body on-device (same inputs/outputs) —
    used only by the benchmark to difference away dispatch overhead."""
    nc = bass.Bass("TRN2", target_bir_lowering=False, debug=False)

    xh_d = nc.dram_tensor("xh", [D, BS], dt.bfloat16, kind="ExternalInput")
    xl_d = nc.dram_tensor("xl", [D, BS], dt.bfloat16, kind="ExternalInput")
    # W columns: 0:128 enc_W hi | 128:256 dec_W hi | 256:384 enc_W lo
    w_d = nc.dram_tensor("w", [D, 384], dt.bfloat16, kind="ExternalInput")
    bias_d = nc.dram_tensor("bias", [128, 256], dt.float32, kind="ExternalInput")
    iota_d = nc.dram_tensor("iota", [128, 128], dt.float32, kind="ExternalInput")
    y_d = nc.dram_tensor("yin", [128, NBT], dt.float32, kind="ExternalInput")

    yhat_d = nc.dram_tensor("yhat", [128, NBT], dt.float32, kind="ExternalOutput")
    idx_d = nc.dram_tensor("idx", [128, NBT], dt.int32, kind="ExternalOutput")
    la_d = nc.dram_tensor("lossacc", [128, 2], dt.float32, kind="ExternalOutput")

    with TileContextFixed(nc) as tc:
        with (
            tc.tile_pool(name="const", bufs=1) as cpool,
            tc.tile_pool(name="xstage", bufs=2) as xpool,
            tc.tile_pool(name="ps", bufs=4, space="PSUM") as pspool,
            tc.tile_pool(name="work", bufs=4) as wpool,
            tc.tile_pool(name="outs", bufs=1) as opool,
        ):
            # ---- constants / weights ----
            w_t = cpool.tile([128, ND * 384], dt.bfloat16, tag="w")
            nc.sync.dma_start(
                w_t[:].rearrange("p (n k) -> p n k", n=ND),
                w_d.ap().rearrange("(n p) k -> p n k", p=128),
            )
            bias_t = cpool.tile([128, 256], dt.float32, tag="bias")
            nc.sync.dma_start(bias_t[:], bias_d.ap()[:])
            iota_t = cpool.tile([128, 128], dt.float32, tag="iota")
            nc.sync.dma_start(iota_t[:], iota_d.ap()[:])
            y_t = cpool.tile([128, NBT], dt.float32, tag="y")
            nc.sync.dma_start(y_t[:], y_d.ap()[:])

            yhat_t = opool.tile([128, NBT], dt.float32, tag="yhat")
            idx_t = opool.tile([128, NBT], dt.int32, tag="idx")
            la_t = opool.tile([128, 2], dt.float32, tag="la")

            def wsl(di, c0, c1):
                return w_t[:, di * 384 + c0 : di * 384 + c1]

            for st in range(NST * reps):
                st = st % NST
                # ---- stage in BW batch columns of x (hi and lo) ----
                xh_t = xpool.tile([128, ND * BW], dt.bfloat16, tag="xh")
                nc.sync.dma_start(
                    xh_t[:].rearrange("p (n b) -> p n b", n=ND),
                    xh_d.ap().rearrange("(n p) b -> p n b", p=128)[
                        :, :, st * BW : (st + 1) * BW
                    ],
                )
                xl_t = xpool.tile([128, ND * BW], dt.bfloat16, tag="xl")
                nc.sync.dma_start(
                    xl_t[:].rearrange("p (n b) -> p n b", n=ND),
                    xl_d.ap().rearrange("(n p) b -> p n b", p=128)[
                        :, :, st * BW : (st + 1) * BW
                    ],
                )
                for bj in range(BW // 128):
                    bt = st * (BW // 128) + bj

                    def xsl(xt, di):
                        return xt[:, di * BW + bj * 128 : di * BW + (bj + 1) * 128]

                    ps = pspool.tile([128, 256], dt.float32, tag="ps")
                    n_mm = 3 * ND
                    i = 0
                    for di in range(ND):
                        # hi @ [Wh_enc | Wh_dec]  (N=256), then hi @ Wl_enc
                        # reusing the same stationary tile (N=128)
                        nc.tensor.matmul(
                            ps[:, 0:256], xsl(xh_t, di), wsl(di, 0, 256),
                            start=(i == 0), stop=False,
                        )
                        i += 1
                        nc.tensor.matmul(
                            ps[:, 0:128], xsl(xh_t, di), wsl(di, 256, 384),
                            start=False, stop=False,
                        )
                        i += 1
                        nc.tensor.matmul(
                            ps[:, 0:128], xsl(xl_t, di), wsl(di, 0, 128),
                            start=False, stop=(i == n_mm - 1),
                        )
                        i += 1

                    # ---- row-wise argmax + select ----
                    stt = wpool.tile([128, 256], dt.float32, tag="stt")
                    nc.vector.tensor_add(stt[:], ps[:], bias_t[:])
                    m8 = wpool.tile([128, 8], dt.float32, tag="m8")
                    nc.vector.max(out=m8[:], in_=stt[:, 0:128])
                    idx8 = wpool.tile([128, 8], dt.uint32, tag="idx8")
                    nc.vector.max_index(out=idx8[:], in_max=m8[:], in_values=stt[:, 0:128])
                    idxf = wpool.tile([128, 1], dt.float32, tag="idxf")
                    nc.vector.tensor_copy(idxf[:], idx8[:, 0:1])
                    onehot = wpool.tile([128, 128], dt.float32, tag="onehot")
                    nc.vector.tensor_scalar(
                        onehot[:], iota_t[:], idxf[:], None,
                        op0=mybir.AluOpType.is_equal,
                    )
                    prod = wpool.tile([128, 128], dt.float32, tag="prod")
                    nc.vector.tensor_mul(prod[:], onehot[:], stt[:, 128:256])
                    nc.vector.reduce_sum(
                        yhat_t[:, bt : bt + 1], prod[:], axis=mybir.AxisListType.X
                    )
                    nc.vector.tensor_copy(idx_t[:, bt : bt + 1], idx8[:, 0:1])

            # ---- loss / accuracy partials ----
            dd = wpool.tile([128, NBT], dt.float32, tag="dd")
            nc.vector.tensor_sub(dd[:], yhat_t[:], y_t[:])
            nc.vector.tensor_mul(dd[:], dd[:], dd[:])
            nc.vector.reduce_sum(la_t[:, 0:1], dd[:], axis=mybir.AxisListType.X)
            sg = wpool.tile([128, NBT], dt.float32, tag="sg")
            nc.vector.tensor_mul(sg[:], yhat_t[:], y_t[:])
            nc.vector.tensor_scalar(
                sg[:], sg[:], 0.0, None, op0=mybir.AluOpType.is_gt
            )
            nc.vector.reduce_sum(la_t[:, 1:2], sg[:], axis=mybir.AxisListType.X)

            nc.sync.dma_start(yhat_d.ap()[:], yhat_t[:])
            nc.sync.dma_start(idx_d.ap()[:], idx_t[:])
            nc.sync.dma_start(la_d.ap()[:], la_t[:])
    return nc


_NC_CACHE = []


def _get_nc():
    if not _NC_CACHE:
        _NC_CACHE.append(_build_nc())
    return _NC_CACHE[0]


def prepare_in_maps(x, y, enc_W, enc_b, dec_W0, dec_bias):
    # ---- host-side layout prep ----
    WT = np.concatenate([enc_W, dec_W0], axis=0).T  # [D, 256] f32
    WTh = WT.astype(ml_dtypes.bfloat16)
    WTl = (WT - WTh.astype(np.float32)).astype(ml_dtypes.bfloat16)
    wfull = np.concatenate([WTh, WTl[:, 0:128]], axis=1)  # [D, 384]
    wfull = np.ascontiguousarray(wfull)

    bias_rep = np.tile(
        np.concatenate([enc_b, dec_bias])[None, :], (128, 1)
    ).astype(np.float32)
    iota_np = np.tile(np.arange(128, dtype=np.float32)[None, :], (128, 1))

    in_maps = []
    for c in range(NCORES):
        xT = x[c * BS : (c + 1) * BS].T  # [D, BS] view
        xh = np.ascontiguousarray(xT.astype(ml_dtypes.bfloat16))
        xl = np.ascontiguousarray(
            (xT - xh.astype(np.float32)).astype(ml_dtypes.bfloat16)
        )
        ysh = np.ascontiguousarray(
            y[c * BS : (c + 1) * BS].reshape(NBT, 128).T
        )
        in_maps.append(
            {
                "xh": xh,
                "xl": xl,
                "w": wfull,
                "bias": bias_rep,
                "iota": iota_np,
                "yin": ysh,
            }
        )
    return in_maps


def kernel(x, y, z, enc_W, enc_b, dec_W, dec_bias, _run_kwargs=None):
    x = np.ascontiguousarray(np.asarray(x, dtype=np.float32))
    y = np.asarray(y, dtype=np.float32)
    enc_W = np.asarray(enc_W, dtype=np.float32)
    enc_b = np.asarray(enc_b, dtype=np.float32)
    dec_W0 = np.asarray(dec_W, dtype=np.float32)[0]
    dec_bias = np.asarray(dec_bias, dtype=np.float32)
    in_maps = prepare_in_maps(x, y, enc_W, enc_b, dec_W0, dec_bias)

    nc = _get_nc()
    res = run_bass_kernel_spmd(
        nc, in_maps, core_ids=list(range(NCORES)), **(_run_kwargs or {})
    )

    y_hat = np.empty(B, dtype=np.float32)
    z_hat = np.empty(B, dtype=np.int32)
    loss_sum = 0.0
    acc_cnt = 0.0
    for c in range(NCORES):
        out = res.results[c]
        y_hat[c * BS : (c + 1) * BS] = out["yhat"].T.reshape(BS)
        z_hat[c * BS : (c + 1) * BS] = out["idx"].T.reshape(BS)
        loss_sum += float(out["lossacc"][:, 0].sum())
        acc_cnt += float(out["lossacc"][:, 1].sum())
    loss = np.float32(loss_sum / B)
    accuracy = np.float32(acc_cnt / B)
    if _run_kwargs is not None:
        kernel.last_result = res
    return (y_hat, z_hat, loss, accuracy)


# revision 12
# speedup vs baseline: 41655.4758x; 2.1869x over previous
"""Trainium2 Bass kernel for nn_DiagnosticNet (topk_masking).

Math (per row b of x [B, D]):
    s[b]     = x[b] @ enc_W.T + enc_b                  # [K]
    idx[b]   = argmax(s[b])
    y_hat[b] = dot(dec_W[0][idx[b]], x[b]) + dec_bias[idx[b]]
    loss     = mean((y_hat - y)^2); accuracy = mean(sign(y_hat) == y)

Strategy: data-parallel over B across 8 cores. On each core, one fused
matmul chain per 128-row tile computes [s | t] where t = x @ dec_W[0].T,
then a row-wise argmax + one-hot select picks y_hat = t[idx] on-chip.
Precision: x is shipped as a bf16 hi/lo pair (same bytes as f32); s uses
the 3-term product (hi*Wh + lo*Wh + hi*Wl) which matches fp32 matmul
error to ~8e-6 at 2x the speed of native fp32 matmuls; t uses hi*Wh
(bf16-level error, fine for y_hat magnitudes). Weights stay replicated.
"""

import sys

import numpy as np

sys.path.insert(0, "/opt/trn_rl_repo")

import ml_dtypes  # noqa: E402
import concourse.bass as bass  # noqa: E402
import concourse.mybir as mybir  # noqa: E402
import concourse.tile as tile  # noqa: E402
from concourse.bass_utils import run_bass_kernel_spmd  # noqa: E402
from concourse.vector_clock import ScopedClock  # noqa: E402

B, D, K = 16384, 2048, 128
NCORES = 8
BS = B // NCORES  # rows per core
NBT = BS // 128  # 128-row tiles per core
ND = D // 128  # 128-deep contraction chunks
BW = 512  # staged batch width (4 tiles per stage)
NST = BS // BW  # stages per core

dt = mybir.dt


def _split_waits(inst):
    """This toolchain's walrus accepts at most one sync-wait command per
    instruction; return carrier NOPs for the excess waits."""
    si = inst.sync_info
    if si is None:
        return []
    waits = si.on_wait
    if not waits or len(waits) <= 1:
        return []
    extras = list(waits[:-1])
    si.on_wait = [waits[-1]]
    nops = []
    for k, w in enumerate(extras):
        nop = mybir.InstNoOp(name=f"{inst.name}-swait{k}", ins=[], outs=[])
        nop.engine = inst.engine
        nop.sync_info = mybir.SyncInfo(on_wait=[w], on_update=[])
        nops.append(nop)
    return nops


class TileContextFixed(tile.TileContext):
    def _add_instruction(self, inst):
        for nop in _split_waits(inst):
            super()._add_instruction(nop)
        super()._add_instruction(inst)

    def _drain_and_barrier(self, tick_clock, wait_clock):
        nc = self.nc
        drain_inst = nc.sync.drain()
        wait_clock.add_sem_waits(
            drain_inst.ins, ScopedClock({None: tick_clock.global_clock})
        )
        si = drain_inst.ins.sync_info
        waits = list(si.on_wait) if si and si.on_wait else []
        if len(waits) > 1:
            si.on_wait = [waits[0]]
            for k, w in enumerate(waits[1:]):
                nop = mybir.InstNoOp(
                    name=f"{drain_inst.ins.name}-dwait{k}", ins=[], outs=[]
                )
                nop.engine = drain_inst.ins.engine
                nop.sync_info = mybir.SyncInfo(on_wait=[w], on_update=[])
                self._add_instruction(nop)
        nc.all_engine_barrier()
        assert self.sems is not None
        popped = nc._tile_sem_poison_stack.pop()
        assert popped is self._sem_poison
        nc.clear_and_free_semaphores(list(self.sems.allocated().values()))
        nc.all_engine_barrier()


def _build_nc(reps=1):
    """reps>1 repeats the whole body on-device (same inputs/outputs) —
    used only by the benchmark to difference away dispatch overhead."""
    nc = bass.Bass("TRN2", target_bir_lowering=False, debug=False)

    # x ships pre-tiled to SBUF layout: [128p, NST, ND, BW] flattened, so
    # every stage DMA reads one fully-contiguous 16 KiB line per partition.
    xh_d = nc.dram_tensor("xh", [128, NST * ND * BW], dt.bfloat16, kind="ExternalInput")
    xl_d = nc.dram_tensor("xl", [128, NST * ND * BW], dt.bfloat16, kind="ExternalInput")
    # W pre-tiled to [128p, ND, 384]; columns of the 384: 0:128 enc_W hi |
    # 128:256 dec_W hi | 256:384 enc_W lo
    w_d = nc.dram_tensor("w", [128, ND * 384], dt.bfloat16, kind="ExternalInput")
    bias_d = nc.dram_tensor("bias", [128, 256], dt.float32, kind="ExternalInput")
    iota_d = nc.dram_tensor("iota", [128, 128], dt.float32, kind="ExternalInput")
    y_d = nc.dram_tensor("yin", [128, NBT], dt.float32, kind="ExternalInput")

    yhat_d = nc.dram_tensor("yhat", [128, NBT], dt.float32, kind="ExternalOutput")
    idx_d = nc.dram_tensor("idx", [128, NBT], dt.int32, kind="ExternalOutput")
    la_d = nc.dram_tensor("lossacc", [128, 2], dt.float32, kind="ExternalOutput")

    with TileContextFixed(nc) as tc:
        with (
            tc.tile_pool(name="const", bufs=1) as cpool,
            tc.tile_pool(name="xstage", bufs=2) as xpool,
            tc.tile_pool(name="ps", bufs=4, space="PSUM") as pspool,
            tc.tile_pool(name="work", bufs=4) as wpool,
            tc.tile_pool(name="outs", bufs=1) as opool,
        ):
            # ---- constants / weights ----
            w_t = cpool.tile([128, ND * 384], dt.bfloat16, tag="w")
            nc.sync.dma_start(w_t[:], w_d.ap()[:])
            bias_t = cpool.tile([128, 256], dt.float32, tag="bias")
            nc.sync.dma_start(bias_t[:], bias_d.ap()[:])
            iota_t = cpool.tile([128, 128], dt.float32, tag="iota")
            nc.sync.dma_start(iota_t[:], iota_d.ap()[:])
            y_t = cpool.tile([128, NBT], dt.float32, tag="y")
            nc.sync.dma_start(y_t[:], y_d.ap()[:])

            yhat_t = opool.tile([128, NBT], dt.float32, tag="yhat")
            idx_t = opool.tile([128, NBT], dt.int32, tag="idx")
            la_t = opool.tile([128, 2], dt.float32, tag="la")

            def wsl(di, c0, c1):
                return w_t[:, di * 384 + c0 : di * 384 + c1]

            for st in range(NST * reps):
                st = st % NST
                # ---- stage in BW batch columns of x (hi and lo) ----
                stw = ND * BW
                xh_t = xpool.tile([128, stw], dt.bfloat16, tag="xh")
                nc.sync.dma_start(xh_t[:], xh_d.ap()[:, st * stw : (st + 1) * stw])
                xl_t = xpool.tile([128, stw], dt.bfloat16, tag="xl")
                nc.sync.dma_start(xl_t[:], xl_d.ap()[:, st * stw : (st + 1) * stw])
                for bj in range(BW // 128):
                    bt = st * (BW // 128) + bj

                    def xsl(xt, di):
                        return xt[:, di * BW + bj * 128 : di * BW + (bj + 1) * 128]

                    ps = pspool.tile([128, 256], dt.float32, tag="ps")
                    n_mm = 3 * ND
                    i = 0
                    for di in range(ND):
                        # hi @ [Wh_enc | Wh_dec]  (N=256), then hi @ Wl_enc
                        # reusing the same stationary tile (N=128)
                        nc.tensor.matmul(
                            ps[:, 0:256], xsl(xh_t, di), wsl(di, 0, 256),
                            start=(i == 0), stop=False,
                        )
                        i += 1
                        nc.tensor.matmul(
                            ps[:, 0:128], xsl(xh_t, di), wsl(di, 256, 384),
                            start=False, stop=False,
                        )
                        i += 1
                        nc.tensor.matmul(
                            ps[:, 0:128], xsl(xl_t, di), wsl(di, 0, 128),
                            start=False, stop=(i == n_mm - 1),
                        )
                        i += 1

                    # ---- row-wise argmax + select ----
                    stt = wpool.tile([128, 256], dt.float32, tag="stt")
                    nc.vector.tensor_add(stt[:], ps[:], bias_t[:])
                    m8 = wpool.tile([128, 8], dt.float32, tag="m8")
                    nc.vector.max(out=m8[:], in_=stt[:, 0:128])
                    idx8 = wpool.tile([128, 8], dt.uint32, tag="idx8")
                    nc.vector.max_index(out=idx8[:], in_max=m8[:], in_values=stt[:, 0:128])
                    idxf = wpool.tile([128, 1], dt.float32, tag="idxf")
                    nc.vector.tensor_copy(idxf[:], idx8[:, 0:1])
                    onehot = wpool.tile([128, 128], dt.float32, tag="onehot")
                    nc.vector.tensor_scalar(
                        onehot[:], iota_t[:], idxf[:], None,
                        op0=mybir.AluOpType.is_equal,
                    )
                    prod = wpool.tile([128, 128], dt.float32, tag="prod")
                    nc.vector.tensor_mul(prod[:], onehot[:], stt[:, 128:256])
                    nc.vector.reduce_sum(
                        yhat_t[:, bt : bt + 1], prod[:], axis=mybir.AxisListType.X
                    )
                    nc.vector.tensor_copy(idx_t[:, bt : bt + 1], idx8[:, 0:1])

            # ---- loss / accuracy partials ----
            dd = wpool.tile([128, NBT], dt.float32, tag="dd")
            nc.vector.tensor_sub(dd[:], yhat_t[:], y_t[:])
            nc.vector.tensor_mul(dd[:], dd[:], dd[:])
            nc.vector.reduce_sum(la_t[:, 0:1], dd[:], axis=mybir.AxisListType.X)
            sg = wpool.tile([128, NBT], dt.float32, tag="sg")
            nc.vector.tensor_mul(sg[:], yhat_t[:], y_t[:])
            nc.vector.tensor_scalar(
                sg[:], sg[:], 0.0, None, op0=mybir.AluOpType.is_gt
            )
            nc.vector.reduce_sum(la_t[:, 1:2], sg[:], axis=mybir.AxisListType.X)

            nc.sync.dma_start(yhat_d.ap()[:], yhat_t[:])
            nc.sync.dma_start(idx_d.ap()[:], idx_t[:])
            nc.sync.dma_start(la_d.ap()[:], la_t[:])
    return nc


_NC_CACHE = []


def _get_nc():
    if not _NC_CACHE:
        _NC_CACHE.append(_build_nc())
    return _NC_CACHE[0]


def prepare_in_maps(x, y, enc_W, enc_b, dec_W0, dec_bias):
    # ---- host-side layout prep ----
    WT = np.concatenate([enc_W, dec_W0], axis=0).T  # [D, 256] f32
    WTh = WT.astype(ml_dtypes.bfloat16)
    WTl = (WT - WTh.astype(np.float32)).astype(ml_dtypes.bfloat16)
    wfull = np.concatenate([WTh, WTl[:, 0:128]], axis=1)  # [D, 384]
    # pre-tile to SBUF layout [128p, ND, 384]
    wfull = np.ascontiguousarray(
        wfull.reshape(ND, 128, 384).transpose(1, 0, 2).reshape(128, ND * 384)
    )

    bias_rep = np.tile(
        np.concatenate([enc_b, dec_bias])[None, :], (128, 1)
    ).astype(np.float32)
    iota_np = np.tile(np.arange(128, dtype=np.float32)[None, :], (128, 1))

    in_maps = []
    for c in range(NCORES):
        xs = x[c * BS : (c + 1) * BS]  # [BS, D]
        # pre-tile to SBUF layout [128p, NST, ND, BW]:
        # element [p, st, n, b] = xs[st*BW + b, n*128 + p]
        x4 = np.ascontiguousarray(
            xs.reshape(NST, BW, ND, 128).transpose(3, 0, 2, 1)
        ).reshape(128, NST * ND * BW)
        xh = x4.astype(ml_dtypes.bfloat16)
        xl = (x4 - xh.astype(np.float32)).astype(ml_dtypes.bfloat16)
        ysh = np.ascontiguousarray(
            y[c * BS : (c + 1) * BS].reshape(NBT, 128).T
        )
        in_maps.append(
            {
                "xh": xh,
                "xl": xl,
                "w": wfull,
                "bias": bias_rep,
                "iota": iota_np,
                "yin": ysh,
            }
        )
    return in_maps


def kernel(x, y, z, enc_W, enc_b, dec_W, dec_bias, _run_kwargs=None):
    x = np.ascontiguousarray(np.asarray(x, dtype=np.float32))
    y = np.asarray(y, dtype=np.float32)
    enc_W = np.asarray(enc_W, dtype=np.float32)
    enc_b = np.asarray(enc_b, dtype=np.float32)
    dec_W0 = np.asarray(dec_W, dtype=np.float32)[0]
    dec_bias = np.asarray(dec_bias, dtype=np.float32)
    in_maps = prepare_in_maps(x, y, enc_W, enc_b, dec_W0, dec_bias)

    nc = _get_nc()
    res = run_bass_kernel_spmd(
        nc, in_maps, core_ids=list(range(NCORES)), **(_run_kwargs or {})
    )

    y_hat = np.empty(B, dtype=np.float32)
    z_hat = np.empty(B, dtype=np.int32)
    loss_sum = 0.0
    acc_cnt = 0.0
    for c in range(NCORES):
        out = res.results[c]
        y_hat[c * BS : (c + 1) * BS] = out["yhat"].T.reshape(BS)
        z_hat[c * BS : (c + 1) * BS] = out["idx"].T.reshape(BS)
        loss_sum += float(out["lossacc"][:, 0].sum())
        acc_cnt += float(out["lossacc"][:, 1].sum())
    loss = np.float32(loss_sum / B)
    accuracy = np.float32(acc_cnt / B)
    if _run_kwargs is not None:
        kernel.last_result = res
    return (y_hat, z_hat, loss, accuracy)


# revision 14
# speedup vs baseline: 53329.2189x; 1.2802x over previous
"""Trainium2 Bass kernel for nn_DiagnosticNet (topk_masking).

Math (per row b of x [B, D]):
    s[b]     = x[b] @ enc_W.T + enc_b                  # [K]
    idx[b]   = argmax(s[b])
    y_hat[b] = dot(dec_W[0][idx[b]], x[b]) + dec_bias[idx[b]]
    loss     = mean((y_hat - y)^2); accuracy = mean(sign(y_hat) == y)

Strategy: data-parallel over B across 8 cores. On each core, one fused
matmul chain per 128-row tile computes [s | t] where t = x @ dec_W[0].T,
then a row-wise argmax + one-hot select picks y_hat = t[idx] on-chip.
Precision: x is shipped as a bf16 hi/lo pair (same bytes as f32); s uses
the 3-term product (hi*Wh + lo*Wh + hi*Wl) which matches fp32 matmul
error to ~8e-6 at 2x the speed of native fp32 matmuls; t uses hi*Wh
(bf16-level error, fine for y_hat magnitudes). Weights stay replicated.
"""

import sys

import numpy as np

sys.path.insert(0, "/opt/trn_rl_repo")

import ml_dtypes  # noqa: E402
import concourse.bass as bass  # noqa: E402
import concourse.mybir as mybir  # noqa: E402
import concourse.tile as tile  # noqa: E402
from concourse.bass_utils import run_bass_kernel_spmd  # noqa: E402
from concourse.vector_clock import ScopedClock  # noqa: E402

B, D, K = 16384, 2048, 128
NCORES = 8
BS = B // NCORES  # rows per core
NBT = BS // 128  # 128-row tiles per core
ND = D // 128  # 128-deep contraction chunks
BW = 512  # staged batch width (4 tiles per stage)
NST = BS // BW  # stages per core

dt = mybir.dt


def _split_waits(inst):
    """This toolchain's walrus accepts at most one sync-wait command per
    instruction; return carrier NOPs for the excess waits."""
    si = inst.sync_info
    if si is None:
        return []
    waits = si.on_wait
    if not waits or len(waits) <= 1:
        return []
    extras = list(waits[:-1])
    si.on_wait = [waits[-1]]
    nops = []
    for k, w in enumerate(extras):
        nop = mybir.InstNoOp(name=f"{inst.name}-swait{k}", ins=[], outs=[])
        nop.engine = inst.engine
        nop.sync_info = mybir.SyncInfo(on_wait=[w], on_update=[])
        nops.append(nop)
    return nops


class TileContextFixed(tile.TileContext):
    def _add_instruction(self, inst):
        for nop in _split_waits(inst):
            super()._add_instruction(nop)
        super()._add_instruction(inst)

    def _drain_and_barrier(self, tick_clock, wait_clock):
        nc = self.nc
        drain_inst = nc.sync.drain()
        wait_clock.add_sem_waits(
            drain_inst.ins, ScopedClock({None: tick_clock.global_clock})
        )
        si = drain_inst.ins.sync_info
        waits = list(si.on_wait) if si and si.on_wait else []
        if len(waits) > 1:
            si.on_wait = [waits[0]]
            for k, w in enumerate(waits[1:]):
                nop = mybir.InstNoOp(
                    name=f"{drain_inst.ins.name}-dwait{k}", ins=[], outs=[]
                )
                nop.engine = drain_inst.ins.engine
                nop.sync_info = mybir.SyncInfo(on_wait=[w], on_update=[])
                self._add_instruction(nop)
        nc.all_engine_barrier()
        assert self.sems is not None
        popped = nc._tile_sem_poison_stack.pop()
        assert popped is self._sem_poison
        nc.clear_and_free_semaphores(list(self.sems.allocated().values()))
        nc.all_engine_barrier()


def _build_nc(reps=1):
    """reps>1 repeats the whole body on-device (same inputs/outputs) —
    used only by the benchmark to difference away dispatch overhead."""
    nc = bass.Bass("TRN2", target_bir_lowering=False, debug=False)

    # x ships pre-tiled to SBUF layout: [128p, NST, ND, BW] flattened, so
    # every stage DMA reads one fully-contiguous 16 KiB line per partition.
    xh_d = nc.dram_tensor("xh", [128, NST * ND * BW], dt.bfloat16, kind="ExternalInput")
    xl_d = nc.dram_tensor("xl", [128, NST * ND * BW], dt.bfloat16, kind="ExternalInput")
    # W pre-tiled to [128p, ND, 384]; columns of the 384: 0:128 enc_W hi |
    # 128:256 dec_W hi | 256:384 enc_W lo
    w_d = nc.dram_tensor("w", [128, ND * 384], dt.bfloat16, kind="ExternalInput")
    bias_d = nc.dram_tensor("bias", [128, 256], dt.float32, kind="ExternalInput")
    iota_d = nc.dram_tensor("iota", [128, 128], dt.float32, kind="ExternalInput")
    y_d = nc.dram_tensor("yin", [128, NBT], dt.float32, kind="ExternalInput")

    yhat_d = nc.dram_tensor("yhat", [128, NBT], dt.float32, kind="ExternalOutput")
    idx_d = nc.dram_tensor("idx", [128, NBT], dt.int32, kind="ExternalOutput")
    la_d = nc.dram_tensor("lossacc", [128, 2], dt.float32, kind="ExternalOutput")

    with TileContextFixed(nc) as tc:
        with (
            tc.tile_pool(name="const", bufs=1) as cpool,
            tc.tile_pool(name="xstage", bufs=2) as xpool,
            tc.tile_pool(name="ps", bufs=8, space="PSUM") as pspool,
            tc.tile_pool(name="work", bufs=8) as wpool,
            tc.tile_pool(name="outs", bufs=1) as opool,
        ):
            # ---- weights: chunk 0 first (unblocks the first matmuls),
            # rest + small constants on the gpsimd ring so they don't
            # queue ahead of x on the sync ring ----
            w_t = cpool.tile([128, ND * 384], dt.bfloat16, tag="w")
            nc.sync.dma_start(w_t[:, 0:768], w_d.ap()[:, 0:768])
            nc.gpsimd.dma_start(w_t[:, 768:], w_d.ap()[:, 768:])
            bias_t = cpool.tile([128, 256], dt.float32, tag="bias")
            nc.gpsimd.dma_start(bias_t[:], bias_d.ap()[:])
            iota_t = cpool.tile([128, 128], dt.float32, tag="iota")
            nc.gpsimd.dma_start(iota_t[:], iota_d.ap()[:])
            y_t = cpool.tile([128, NBT], dt.float32, tag="y")
            nc.gpsimd.dma_start(y_t[:], y_d.ap()[:])

            yhat_t = opool.tile([128, NBT], dt.float32, tag="yhat")
            idx_t = opool.tile([128, NBT], dt.int32, tag="idx")
            la_t = opool.tile([128, 2], dt.float32, tag="la")

            def wsl(di, c0, c1):
                return w_t[:, di * 384 + c0 : di * 384 + c1]

            NBJ = BW // 128
            stw = ND * BW
            for st in range(NST * reps):
                st = st % NST
                # ---- stage in BW batch columns of x (hi and lo) ----
                xh_t = xpool.tile([128, stw], dt.bfloat16, tag="xh")
                xl_t = xpool.tile([128, stw], dt.bfloat16, tag="xl")
                if st == 0:
                    # chunk-granular first stage so matmuls start after
                    # ~0.5 MB instead of the full 4 MB
                    for d0 in range(0, ND, 2):
                        sl = slice(d0 * BW, (d0 + 2) * BW)
                        nc.sync.dma_start(xh_t[:, sl], xh_d.ap()[:, sl])
                        nc.sync.dma_start(xl_t[:, sl], xl_d.ap()[:, sl])
                else:
                    off = st * stw
                    nc.sync.dma_start(
                        xh_t[:], xh_d.ap()[:, off : off + stw]
                    )
                    nc.sync.dma_start(
                        xl_t[:], xl_d.ap()[:, off : off + stw]
                    )

                # ---- d-outer matmuls: NBJ concurrent PSUM chains ----
                ps_tiles = [
                    pspool.tile([128, 256], dt.float32, tag="ps", name=f"ps_{st}_{r}")
                    for r in range(NBJ)
                ]
                for di in range(ND):
                    for bj in range(NBJ):
                        xsl = slice(di * BW + bj * 128, di * BW + (bj + 1) * 128)
                        ps = ps_tiles[bj]
                        nc.tensor.matmul(
                            ps[:, 0:256], xh_t[:, xsl], wsl(di, 0, 256),
                            start=(di == 0), stop=False,
                        )
                        nc.tensor.matmul(
                            ps[:, 0:128], xh_t[:, xsl], wsl(di, 256, 384),
                            start=False, stop=False,
                        )
                        nc.tensor.matmul(
                            ps[:, 0:128], xl_t[:, xsl], wsl(di, 0, 128),
                            start=False, stop=(di == ND - 1),
                        )

                # ---- row-wise argmax + select per tile ----
                for bj in range(NBJ):
                    bt = st * NBJ + bj
                    ps = ps_tiles[bj]
                    stt = wpool.tile([128, 256], dt.float32, tag="stt")
                    nc.vector.tensor_add(stt[:], ps[:], bias_t[:])
                    m8 = wpool.tile([128, 8], dt.float32, tag="m8")
                    nc.vector.max(out=m8[:], in_=stt[:, 0:128])
                    idx8 = wpool.tile([128, 8], dt.uint32, tag="idx8")
                    nc.vector.max_index(out=idx8[:], in_max=m8[:], in_values=stt[:, 0:128])
                    idxf = wpool.tile([128, 1], dt.float32, tag="idxf")
                    nc.vector.tensor_copy(idxf[:], idx8[:, 0:1])
                    onehot = wpool.tile([128, 128], dt.float32, tag="onehot")
                    nc.vector.tensor_scalar(
                        onehot[:], iota_t[:], idxf[:], None,
                        op0=mybir.AluOpType.is_equal,
                    )
                    prod = wpool.tile([128, 128], dt.float32, tag="prod")
                    nc.vector.tensor_mul(prod[:], onehot[:], stt[:, 128:256])
                    nc.vector.reduce_sum(
                        yhat_t[:, bt : bt + 1], prod[:], axis=mybir.AxisListType.X
                    )
                    nc.vector.tensor_copy(idx_t[:, bt : bt + 1], idx8[:, 0:1])

            # ---- loss / accuracy partials ----
            dd = wpool.tile([128, NBT], dt.float32, tag="dd")
            nc.vector.tensor_sub(dd[:], yhat_t[:], y_t[:])
            nc.vector.tensor_mul(dd[:], dd[:], dd[:])
            nc.vector.reduce_sum(la_t[:, 0:1], dd[:], axis=mybir.AxisListType.X)
            sg = wpool.tile([128, NBT], dt.float32, tag="sg")
            nc.vector.tensor_mul(sg[:], yhat_t[:], y_t[:])
            nc.vector.tensor_scalar(
                sg[:], sg[:], 0.0, None, op0=mybir.AluOpType.is_gt
            )
            nc.vector.reduce_sum(la_t[:, 1:2], sg[:], axis=mybir.AxisListType.X)

            nc.sync.dma_start(yhat_d.ap()[:], yhat_t[:])
            nc.sync.dma_start(idx_d.ap()[:], idx_t[:])
            nc.sync.dma_start(la_d.ap()[:], la_t[:])
    return nc


_NC_CACHE = []


def _get_nc():
    if not _NC_CACHE:
        _NC_CACHE.append(_build_nc())
    return _NC_CACHE[0]


def prepare_in_maps(x, y, enc_W, enc_b, dec_W0, dec_bias):
    # ---- host-side layout prep ----
    WT = np.concatenate([enc_W, dec_W0], axis=0).T  # [D, 256] f32
    WTh = WT.astype(ml_dtypes.bfloat16)
    WTl = (WT - WTh.astype(np.float32)).astype(ml_dtypes.bfloat16)
    wfull = np.concatenate([WTh, WTl[:, 0:128]], axis=1)  # [D, 384]
    # pre-tile to SBUF layout [128p, ND, 384]
    wfull = np.ascontiguousarray(
        wfull.reshape(ND, 128, 384).transpose(1, 0, 2).reshape(128, ND * 384)
    )

    bias_rep = np.tile(
        np.concatenate([enc_b, dec_bias])[None, :], (128, 1)
    ).astype(np.float32)
    iota_np = np.tile(np.arange(128, dtype=np.float32)[None, :], (128, 1))

    in_maps = []
    for c in range(NCORES):
        xs = x[c * BS : (c + 1) * BS]  # [BS, D]
        # pre-tile to SBUF layout [128p, NST, ND, BW]:
        # element [p, st, n, b] = xs[st*BW + b, n*128 + p]
        x4 = np.ascontiguousarray(
            xs.reshape(NST, BW, ND, 128).transpose(3, 0, 2, 1)
        ).reshape(128, NST * ND * BW)
        xh = x4.astype(ml_dtypes.bfloat16)
        xl = (x4 - xh.astype(np.float32)).astype(ml_dtypes.bfloat16)
        ysh = np.ascontiguousarray(
            y[c * BS : (c + 1) * BS].reshape(NBT, 128).T
        )
        in_maps.append(
            {
                "xh": xh,
                "xl": xl,
                "w": wfull,
                "bias": bias_rep,
                "iota": iota_np,
                "yin": ysh,
            }
        )
    return in_maps


def kernel(x, y, z, enc_W, enc_b, dec_W, dec_bias, _run_kwargs=None):
    x = np.ascontiguousarray(np.asarray(x, dtype=np.float32))
    y = np.asarray(y, dtype=np.float32)
    enc_W = np.asarray(enc_W, dtype=np.float32)
    enc_b = np.asarray(enc_b, dtype=np.float32)
    dec_W0 = np.asarray(dec_W, dtype=np.float32)[0]
    dec_bias = np.asarray(dec_bias, dtype=np.float32)
    in_maps = prepare_in_maps(x, y, enc_W, enc_b, dec_W0, dec_bias)

    nc = _get_nc()
    res = run_bass_kernel_spmd(
        nc, in_maps, core_ids=list(range(NCORES)), **(_run_kwargs or {})
    )

    y_hat = np.empty(B, dtype=np.float32)
    z_hat = np.empty(B, dtype=np.int32)
    loss_sum = 0.0
    acc_cnt = 0.0
    for c in range(NCORES):
        out = res.results[c]
        y_hat[c * BS : (c + 1) * BS] = out["yhat"].T.reshape(BS)
        z_hat[c * BS : (c + 1) * BS] = out["idx"].T.reshape(BS)
        loss_sum += float(out["lossacc"][:, 0].sum())
        acc_cnt += float(out["lossacc"][:, 1].sum())
    loss = np.float32(loss_sum / B)
    accuracy = np.float32(acc_cnt / B)
    if _run_kwargs is not None:
        kernel.last_result = res
    return (y_hat, z_hat, loss, accuracy)
